# revision 1
# baseline (speedup 1.0000x reference)
"""Trainium2 Bass kernel for nn_AxialBlock (axial attention, branches W/H/T).

Self-contained: accepts FULL inputs as in reference.setup_inputs(), shards
across 8 NeuronCores as (batch x head-half), runs one SPMD Bass program,
gathers on host.

Hardcoded problem shape: x (4, 512, 16, 32, 32) f32, C=512, 8 heads, d=64.

Per-core layout: activations channel-major [C, tokens], all branch work done
on a tile whose free (token) order makes that branch's sequences contiguous:
W uses the natural (h, w) plane order, H a (w, h) reorder, T a (w, t)
reorder of an (t, w) h-row tile. QKV projections and attention matmuls in
bf16 (fp32 PSUM), output projections in fp32r. The tiny per-sequence
attention runs as tile_position-packed PE matmuls over 128-token groups
(4x32-token col strips), two PSUM score banks keyed by head row-group.

Phase 1 (per t-plane): W + H branches, partial Y summed to DRAM scratch.
Phase 2 (per h-row): T branch + combine with scratch + store.
"""

import numpy as np

import concourse.bass as bass
import concourse.mybir as mybir
from concourse import bacc, tile
from concourse.bass_utils import run_bass_kernel_spmd

F32 = mybir.dt.float32
F32R = mybir.dt.float32r
BF16 = mybir.dt.bfloat16
AF = mybir.ActivationFunctionType

B, C, T, H, W = 4, 512, 16, 32, 32
NH, D = 8, 64
HH = 4  # heads per core (head-half)
CH = HH * D  # 256 channels per core
NEG = -30000.0


def build_nc():
    nc = bacc.Bacc("TRN2", target_bir_lowering=False, debug=False, num_devices=8)

    x_in = nc.dram_tensor("x_in", [C, T, H, W], F32, kind="ExternalInput")
    wqkv = {
        ax: nc.dram_tensor(f"wqkv_{ax}", [C, 3 * CH], F32, kind="ExternalInput")
        for ax in ("w", "h", "t")
    }
    fc = {
        ax: nc.dram_tensor(f"fc_{ax}", [CH, C], F32, kind="ExternalInput")
        for ax in ("w", "h", "t")
    }
    y_out = nc.dram_tensor("y_out", [C, T, H, W], F32, kind="ExternalOutput")
    y_wh = nc.dram_tensor("y_wh", [T, C, H * W], F32, kind="Internal")

    # T-branch pair mask, duplicated over the chunk dim: sc bank layout is
    # [128 (4s x 32l), 2 (c), 32 (m)]; within a col strip the two 16-token
    # sequences must not attend to each other.
    mrows = np.arange(128) % 32
    mcols = np.arange(32)
    m2 = np.where((mrows[:, None] // 16) == (mcols[None, :] // 16), 0.0, NEG)
    mask_np = np.stack([m2, m2], axis=1).astype(np.float32)  # [128, 2, 32]
    mask_dram = nc.inline_tensor(mask_np, name="tmask")

    with tile.TileContext(nc) as tc:
        with (
            tc.tile_pool(name="consts", bufs=1) as consts,
            tc.tile_pool(name="xtp", bufs=2) as xtp,
            tc.tile_pool(name="qkv", bufs=2) as qkvp,
            tc.tile_pool(name="att", bufs=2) as attp,
            tc.tile_pool(name="yp", bufs=2) as yp,
            tc.tile_pool(name="ps", bufs=1, space="PSUM") as ps,
        ):
            w_t = {}
            fc_t = {}
            for ax in ("w", "h", "t"):
                w_t[ax] = consts.tile([128, 4, 3 * CH], BF16, name=f"w_{ax}")
                nc.gpsimd.dma_start(
                    out=w_t[ax],
                    in_=wqkv[ax].rearrange("(kc kp) m -> kp kc m", kp=128),
                )
                fc_t[ax] = consts.tile([128, 2, C], F32R, name=f"fc_{ax}")
                nc.sync.dma_start(
                    out=fc_t[ax],
                    in_=fc[ax].rearrange("(kc kp) m -> kp kc m", kp=128)
                    .bitcast(F32R),
                )
            mask_t = consts.tile([128, 2, 32], F32, name="mask_t")
            nc.sync.dma_start(out=mask_t, in_=mask_dram[:, :, :])

            def qkv_project(xt, ax, ntok, qt, kt, vr):
                """xt [128, 4, ntok] bf16 (seq-contiguous order) ->
                qt/kt [128, 2, ntok] bf16, vr [128, ngrp, 256] bf16."""
                for mc in range(4):  # q0 q1 k0 k1 output chunks
                    dst = qt if mc < 2 else kt
                    oc = mc % 2
                    for tt in range(ntok // 512):
                        acc = ps.tile([128, 512], F32, name="acc", tag="big",
                                      bufs=2)
                        for ic in range(4):
                            nc.tensor.matmul(
                                acc,
                                w_t[ax][:, ic, mc * 128 : (mc + 1) * 128],
                                xt[:, ic, tt * 512 : (tt + 1) * 512],
                                start=(ic == 0),
                                stop=(ic == 3),
                            )
                        nc.scalar.copy(
                            out=dst[:, oc, tt * 512 : (tt + 1) * 512], in_=acc
                        )
                for g in range(ntok // 128):
                    acc = ps.tile([128, 256], F32, name="acc2", tag="big",
                                  bufs=2)
                    for ic in range(4):
                        nc.tensor.matmul(
                            acc,
                            xt[:, ic, g * 128 : (g + 1) * 128],
                            w_t[ax][:, ic, 512:768],
                            start=(ic == 0),
                            stop=(ic == 3),
                        )
                    nc.scalar.copy(out=vr[:, g, :], in_=acc)

            def attention(ax, ntok, qt, kt, vr, ot):
                for g in range(ntok // 128):
                    gs = slice(g * 128, (g + 1) * 128)
                    sc = [
                        ps.tile([128, 2, 32], F32, name=f"sc{h}",
                                tag=f"sc{h}", bufs=2)
                        for h in range(2)
                    ]
                    # scores: s x c x h; h innermost alternates PE row group
                    for s in range(4):
                        q32 = slice(g * 128 + s * 32, g * 128 + (s + 1) * 32)
                        for c in range(2):
                            for h in range(2):
                                nc.tensor.matmul(
                                    sc[h][s * 32 : (s + 1) * 32, c, :],
                                    qt[h * 64 : (h + 1) * 64, c, q32],
                                    kt[h * 64 : (h + 1) * 64, c, q32],
                                    start=True,
                                    stop=True,
                                    tile_position=(h * 64, s * 32),
                                    skip_group_check=True,
                                )
                    if ax == "t":
                        for h in range(2):
                            nc.vector.tensor_add(
                                out=sc[h], in0=sc[h], in1=mask_t
                            )
                    att = attp.tile([128, 2, 2, 32], BF16, name="att",
                                    tag="att", bufs=2)
                    rs = attp.tile([128, 4], F32, name="rs", tag="rs", bufs=2)
                    rv = attp.tile([128, 4], F32, name="rv", tag="rv", bufs=2)
                    for h in range(2):
                        nc.scalar.activation(
                            out=att[:, :, h, :], in_=sc[h], func=AF.Exp
                        )
                        nc.vector.tensor_reduce(
                            out=rs[:, 2 * h : 2 * h + 2],
                            in_=att[:, :, h, :],
                            axis=mybir.AxisListType.X,
                            op=mybir.AluOpType.add,
                        )
                    nc.vector.reciprocal(out=rv, in_=rs)
                    for c in range(2):
                        for h in range(2):
                            nc.vector.tensor_scalar_mul(
                                out=att[:, c, h, :],
                                in0=att[:, c, h, :],
                                scalar1=rv[:, 2 * h + c : 2 * h + c + 1],
                            )
                    attT = attp.tile([128, 2, 2, 32], BF16, name="attT",
                                     tag="attT", bufs=2)
                    nc.vector.transpose(
                        out=attT.rearrange("p c h n -> p (c h n)"),
                        in_=att.rearrange("p c h n -> p (c h n)"),
                    )
                    for s in range(4):
                        av = ps.tile([128, 2, 32], F32, name=f"av{s % 2}",
                                     tag=f"av{s % 2}", bufs=1)
                        for c in range(2):
                            for h in range(2):
                                nc.tensor.matmul(
                                    av[h * 64 : (h + 1) * 64, c, :],
                                    vr[s * 32 : (s + 1) * 32, g,
                                       (2 * c + h) * 64 : (2 * c + h + 1) * 64],
                                    attT[s * 32 : (s + 1) * 32, c, h, :],
                                    start=True,
                                    stop=True,
                                    tile_position=(s * 32, h * 64),
                                    skip_group_check=True,
                                )
                        nc.scalar.copy(
                            out=ot[:, :, g * 128 + s * 32 : g * 128 + (s + 1) * 32],
                            in_=av,
                        )

            def out_project(ax, ntok, ot, write_fn):
                for tt in range(ntok // 512):
                    for oc in range(4):
                        yps = ps.tile([128, 512], F32, name="yps", tag="big",
                                      bufs=2)
                        for ic in range(2):
                            nc.tensor.matmul(
                                yps,
                                fc_t[ax][:, ic, oc * 128 : (oc + 1) * 128],
                                ot[:, ic, tt * 512 : (tt + 1) * 512],
                                start=(ic == 0),
                                stop=(ic == 1),
                            )
                        write_fn(oc, tt, yps)

            # ---------------- Phase 1: W + H branches per t-plane
            for p in range(T):
                xt = xtp.tile([128, 4, 1024], BF16, name="xt", tag="xt", bufs=2)
                for cc in range(4):
                    nc.gpsimd.dma_start(
                        out=xt[:, cc, :],
                        in_=x_in[cc * 128 : (cc + 1) * 128, p, :, :]
                        .rearrange("p h w -> p (h w)"),
                    )
                xth = xtp.tile([128, 4, 1024], BF16, name="xth", tag="xth",
                               bufs=2)
                nc.vector.tensor_copy(
                    out=xth.rearrange("p c (w h) -> p c w h", h=32),
                    in_=xt.rearrange("p c (h w) -> p c w h", w=32),
                )
                ysb = yp.tile([128, 4, 1024], F32, name="ysb", tag="ysb", bufs=2)
                for bi, ax in enumerate(("w", "h")):
                    xb = xt if ax == "w" else xth
                    qt = qkvp.tile([128, 2, 1024], BF16, name="qt", tag="qt",
                                   bufs=2)
                    kt = qkvp.tile([128, 2, 1024], BF16, name="kt", tag="kt",
                                   bufs=2)
                    vr = qkvp.tile([128, 8, 256], BF16, name="vr", tag="vr",
                                   bufs=2)
                    ot = qkvp.tile([128, 2, 1024], F32R, name="ot", tag="ot",
                                   bufs=2)
                    qkv_project(xb, ax, 1024, qt, kt, vr)
                    attention(ax, 1024, qt, kt, vr, ot)
                    if ax == "w":
                        def wr(oc, tt, yps):
                            nc.scalar.copy(
                                out=ysb[:, oc, tt * 512 : (tt + 1) * 512],
                                in_=yps)
                    else:
                        def wr(oc, tt, yps):
                            dv = ysb[:, oc, :].rearrange(
                                "p (h w) -> p w h", w=32)[:, 16 * tt : 16 * (tt + 1), :]
                            nc.vector.tensor_tensor(
                                out=dv,
                                in0=yps.rearrange("p (w h) -> p w h", h=32),
                                in1=dv,
                                op=mybir.AluOpType.add,
                            )
                    out_project(ax, 1024, ot, wr)
                for cc in range(4):
                    nc.sync.dma_start(
                        out=y_wh[p, cc * 128 : (cc + 1) * 128, :],
                        in_=ysb[:, cc, :],
                    )

            # ---------------- Phase 2: T branch per h-row (+ combine)
            for r in range(H):
                xn = xtp.tile([128, 4, 512], BF16, name="xn", tag="xt", bufs=2)
                for cc in range(4):
                    nc.gpsimd.dma_start(
                        out=xn[:, cc, :].rearrange("p (t w) -> p t w", t=16),
                        in_=x_in[cc * 128 : (cc + 1) * 128, :, r, :],
                    )
                xt = xtp.tile([128, 4, 512], BF16, name="xtt", tag="xth",
                              bufs=2)
                nc.vector.tensor_copy(
                    out=xt.rearrange("p c (w t) -> p c w t", t=16),
                    in_=xn.rearrange("p c (t w) -> p c w t", w=32),
                )
                ywh = yp.tile([128, 4, 512], F32, name="ywh", tag="ywh", bufs=2)
                for cc in range(4):
                    nc.sync.dma_start(
                        out=ywh[:, cc, :].rearrange("p (t w) -> p t w", t=16),
                        in_=y_wh[:, cc * 128 : (cc + 1) * 128,
                                 r * 32 : (r + 1) * 32].rearrange(
                                     "t p w -> p t w"),
                    )
                qt = qkvp.tile([128, 2, 512], BF16, name="qt2", tag="qt",
                               bufs=2)
                kt = qkvp.tile([128, 2, 512], BF16, name="kt2", tag="kt",
                               bufs=2)
                vr = qkvp.tile([128, 4, 256], BF16, name="vr2", tag="vr",
                               bufs=2)
                ot = qkvp.tile([128, 2, 512], F32R, name="ot2", tag="ot",
                               bufs=2)
                qkv_project(xt, "t", 512, qt, kt, vr)
                attention("t", 512, qt, kt, vr, ot)
                ysb = yp.tile([128, 4, 512], F32, name="ysb2", tag="ysb",
                              bufs=2)

                def wr2(oc, tt, yps):
                    # yps free order (w, t); ysb natural (t, w)
                    nc.vector.tensor_tensor(
                        out=ysb[:, oc, :].rearrange("p (t w) -> p w t", w=32),
                        in0=yps.rearrange("p (w t) -> p w t", t=16),
                        in1=ywh[:, oc, :].rearrange("p (t w) -> p w t", w=32),
                        op=mybir.AluOpType.add,
                    )

                out_project("t", 512, ot, wr2)
                for cc in range(4):
                    nc.sync.dma_start(
                        out=y_out[cc * 128 : (cc + 1) * 128, :, r, :],
                        in_=ysb[:, cc, :].rearrange("p (t w) -> p t w", t=16),
                    )
    nc.compile()
    return nc


_NC_CACHE = {}


def _get_nc():
    if "nc" not in _NC_CACHE:
        _NC_CACHE["nc"] = build_nc()
    return _NC_CACHE["nc"]


def kernel(x, wq_w, wk_w, wv_w, fc_w, fb_w,
           wq_h, wk_h, wv_h, fc_h, fb_h,
           wq_t, wk_t, wv_t, fc_t, fb_t, _trace=False):
    x = np.asarray(x, np.float32)
    scale = 1.0 / np.sqrt(np.float32(D))
    branches = {
        "w": (np.asarray(wq_w, np.float32), np.asarray(wk_w, np.float32),
              np.asarray(wv_w, np.float32), np.asarray(fc_w, np.float32)),
        "h": (np.asarray(wq_h, np.float32), np.asarray(wk_h, np.float32),
              np.asarray(wv_h, np.float32), np.asarray(fc_h, np.float32)),
        "t": (np.asarray(wq_t, np.float32), np.asarray(wk_t, np.float32),
              np.asarray(wv_t, np.float32), np.asarray(fc_t, np.float32)),
    }
    fb_sum = (np.asarray(fb_w, np.float32) + np.asarray(fb_h, np.float32)
              + np.asarray(fb_t, np.float32))

    in_maps = []
    for core in range(8):
        b, hh = core // 2, core % 2
        m = {"x_in": np.ascontiguousarray(x[b])}
        cols = slice(hh * CH, (hh + 1) * CH)
        for ax, (wq, wk, wv, fcm) in branches.items():
            m[f"wqkv_{ax}"] = np.ascontiguousarray(
                np.concatenate(
                    [wq[:, cols] * scale, wk[:, cols], wv[:, cols]], axis=1
                )
            )
            m[f"fc_{ax}"] = np.ascontiguousarray(fcm[cols, :])
        in_maps.append(m)

    nc = _get_nc()
    res = run_bass_kernel_spmd(
        nc, in_maps, core_ids=list(range(8)), trace=_trace,
    )
    y = np.empty((B, C, T, H, W), np.float32)
    for b in range(B):
        y[b] = res.results[2 * b]["y_out"] + res.results[2 * b + 1]["y_out"]
    y += fb_sum[None, :, None, None, None]
    if _trace:
        _NC_CACHE["last_result"] = res
    return y

